# revision 6
# baseline (speedup 1.0000x reference)
"""MoE top-1 routing kernel for 8 TRN2 NeuronCores (expert parallelism).

Self-contained: takes full inputs, shards experts across 8 cores, returns the
full output (host sums the 8 disjoint per-expert partials).

v5 design (single-collective routing):
- Gating token-sharded (fp32 shard matmul + DVE argmax/softmax).
- Each core computes LOCAL queue positions for its own 1024 tokens (one
  triangular matmul + a tiny offset matmul), then scatters (token_id+1, gate)
  at row expert*1024 + local_pos into a zero-prefilled [T, 2] buffer
  (8 indirect DMAs, all before any collective).
- ONE AllToAll routes expert region e to core e. The receiver derives the
  per-shard counts by counting nonzero rows, turns them into shard base
  offsets (two tiny matmuls), computes for each of its 1280 capacity slots
  the source row (shard, local pos), and compacts with 10 small indirect
  gathers. Capacity dropping falls out of gathering only slots [0, C).
- FFN in bf16, w2 resident, w1 streamed twice over two 640-slot halves,
  fused bias+ReLU on ACT, gate-scaled rows scattered into pre-zeroed output.
"""
import numpy as np
import ml_dtypes
from contextlib import ExitStack

import concourse.bass as bass
import concourse.tile as tile
from concourse import bacc, mybir
from concourse.bass_utils import run_bass_kernel_spmd

dt = mybir.dt

B, S, M, E, DFF = 4, 2048, 1024, 8, 4096
T = B * S                  # 8192 tokens
C = int(1.25 * T / E)      # 1280 capacity
P = 128
NT = T // P                # 64 token tiles
MC = M // P                # 8 m chunks
DC = DFF // P              # 32 dff chunks
SCN = C // P               # 10 slot chunks
HALF = C // 2              # 640 slots per half
TSH = T // E               # 1024 tokens per shard
LT = TSH // P              # 8 local tiles
LE = LT * E                # 64 (ti, e) columns
JS = SCN * E               # 80 (sc, j) columns
BIG = 1.0e9

_CACHE = {}


def _build_nc(stage=5):
    nc = bacc.Bacc("TRN2", target_bir_lowering=False, debug=False)

    # ---- I/O ----
    xTs = nc.dram_tensor("xTs", [M, TSH], dt.float32, kind="ExternalInput")
    xb = nc.dram_tensor("xb", [T, M], dt.bfloat16, kind="ExternalInput")
    wg = nc.dram_tensor("wg", [M, E], dt.float32, kind="ExternalInput")
    w1p = nc.dram_tensor("w1p", [DC, P, MC, P], dt.bfloat16, kind="ExternalInput")
    w2p = nc.dram_tensor("w2p", [P, DC, M], dt.bfloat16, kind="ExternalInput")
    b1v = nc.dram_tensor("b1v", [DFF], dt.float32, kind="ExternalInput")
    b2b = nc.dram_tensor("b2b", [P, M], dt.float32, kind="ExternalInput")
    eiota = nc.dram_tensor("eiota", [P, LT, E], dt.float32, kind="ExternalInput")
    triu = nc.dram_tensor("triu", [P, P], dt.float32, kind="ExternalInput")
    identf = nc.dram_tensor("identf", [P, P], dt.float32, kind="ExternalInput")
    identb = nc.dram_tensor("identb", [P, P], dt.bfloat16, kind="ExternalInput")
    w64d = nc.dram_tensor("w64d", [LE, LE], dt.float32, kind="ExternalInput")
    wtri8d = nc.dram_tensor("wtri8d", [E, JS], dt.float32, kind="ExternalInput")
    ecbd = nc.dram_tensor("ecbd", [P, LE], dt.float32, kind="ExternalInput")
    sidxd = nc.dram_tensor("sidxd", [P, JS], dt.float32, kind="ExternalInput")
    slotdd = nc.dram_tensor("slotdd", [P, SCN], dt.float32, kind="ExternalInput")
    tokp1d = nc.dram_tensor("tokp1d", [P, LT], dt.float32, kind="ExternalInput")
    outd = nc.dram_tensor("out", [T, M], dt.float32, kind="ExternalOutput")

    # ---- internal DRAM ----
    igd_loc = nc.dram_tensor("igd_loc", [T, 2], dt.float32)
    igd_rcv = nc.dram_tensor("igd_rcv", [T, 2], dt.float32)

    with tile.TileContext(nc) as tc, ExitStack() as ctx:
        sb = ctx.enter_context(tc.tile_pool(name="sb", bufs=1))
        sbx = ctx.enter_context(tc.tile_pool(name="sbx", bufs=9))   # x stream
        sbw1 = ctx.enter_context(tc.tile_pool(name="sbw1", bufs=4))  # w1 stream
        sbst = ctx.enter_context(tc.tile_pool(name="sbst", bufs=3))  # staging
        sbr = ctx.enter_context(tc.tile_pool(name="sbr", bufs=2))   # routing small

        # ---------- persistent tiles; gating inputs first on sync queue ----
        wgt = sb.tile([P, MC * E], dt.float32)       # gate weights (mc, e)
        nc.sync.dma_start(wgt[:], wg[:].rearrange("(mc p) e -> p mc e", p=P))
        xts = {}
        for blk in range(TSH // 512):
            for k in range(MC):
                xt = sbx.tile([P, 512], dt.float32, tag="xt")
                nc.sync.dma_start(
                    xt[:], xTs[k * P:(k + 1) * P, blk * 512:(blk + 1) * 512])
                xts[(blk, k)] = xt
        eit = sb.tile([P, LE], dt.float32)
        nc.sync.dma_start(eit[:], eiota[:])
        trit = sb.tile([P, P], dt.float32)
        nc.sync.dma_start(trit[:], triu[:])
        idf = sb.tile([P, P], dt.float32)
        nc.sync.dma_start(idf[:], identf[:])
        w64t = sb.tile([LE, LE], dt.float32)
        nc.sync.dma_start(w64t[:], w64d[:])
        wtri8t = sb.tile([E, JS], dt.float32)
        nc.sync.dma_start(wtri8t[:], wtri8d[:])
        ecbt = sb.tile([P, LE], dt.float32)
        nc.sync.dma_start(ecbt[:], ecbd[:])
        sidxt = sb.tile([P, JS], dt.float32)
        nc.sync.dma_start(sidxt[:], sidxd[:])
        slotdt = sb.tile([P, SCN], dt.float32)
        nc.sync.dma_start(slotdt[:], slotdd[:])
        tokp1 = sb.tile([P, LT], dt.float32)
        nc.sync.dma_start(tokp1[:], tokp1d[:])
        idb = sb.tile([P, P], dt.bfloat16)
        nc.sync.dma_start(idb[:], identb[:])
        b1t = sb.tile([P, DC], dt.float32)           # b1 per-partition cols
        nc.sync.dma_start(b1t[:], b1v[:].rearrange("(d p) -> p d", p=P))
        # w2 resident, loaded on sync queue after the gating-critical inputs
        w2t = sb.tile([P, DC * M], dt.bfloat16)      # resident w2 (d, m)
        if stage >= 5:
            for q in range(4):
                nc.sync.dma_start(
                    w2t[:, q * 8 * M:(q + 1) * 8 * M],
                    w2p[:, q * 8:(q + 1) * 8, :])

        ones1 = sb.tile([1, P], dt.float32)
        nc.gpsimd.memset(ones1[:], 1.0)
        onescol = sb.tile([P, 1], dt.float32)
        nc.gpsimd.memset(onescol[:], 1.0)
        nines = sb.tile([P, LE], dt.float32)
        nc.gpsimd.memset(nines[:], 9.0)
        huget = sb.tile([P, LE], dt.float32)
        nc.gpsimd.memset(huget[:], BIG)
        bigt = sb.tile([P, SCN], dt.float32)
        nc.gpsimd.memset(bigt[:], 1.5e9)
        zpre = sb.tile([P, T * 2 // P], dt.float32)  # [128, 128]
        nc.vector.memset(zpre[:], 0.0)
        # zero-prefill the local slot buffer (any layout; it's all zeros)
        nc.scalar.dma_start(
            igd_loc[:].rearrange("(p c) two -> p c two", p=P), zpre[:])
        b2t = sb.tile([P, M], dt.float32)
        nc.scalar.dma_start(b2t[:], b2b[:])

        eg_stk = sb.tile([P, LT * 2], dt.float32)    # local (eidx, gate) cols
        idx_t = sb.tile([P, SCN], dt.int32)
        idxf = sb.tile([P, SCN], dt.float32)
        gate_f = sb.tile([P, SCN], dt.float32)

        # ---------- phase A: sharded gating, batched routing ----------
        lg_stk = sb.tile([P, LE], dt.float32)
        with tc.tile_pool(name="psg", bufs=4, space="PSUM") as psg:
            lgT = sb.tile([8, TSH], dt.float32)
            for blk in range(TSH // 512):
                pl = psg.tile([8, 512], dt.float32, tag="pl")
                for k in range(MC):
                    nc.tensor.matmul(
                        pl[:], lhsT=wgt[:, k * E:(k + 1) * E],
                        rhs=xts[(blk, k)][:],
                        start=(k == 0), stop=(k == MC - 1))
                nc.vector.tensor_copy(lgT[:, blk * 512:(blk + 1) * 512], pl[:])
            for ti in range(LT):
                pq = psg.tile([P, E], dt.float32, tag="pq")
                nc.tensor.transpose(
                    out=pq[:], in_=lgT[:, ti * P:(ti + 1) * P],
                    identity=idf[:8, :8])
                nc.vector.tensor_copy(lg_stk[:, ti * E:(ti + 1) * E], pq[:])
        lg3 = lg_stk[:].rearrange("p (ti e) -> p ti e", e=E)
        mx_stk = sb.tile([P, LT], dt.float32)
        nc.vector.tensor_reduce(
            out=mx_stk[:], in_=lg3, axis=mybir.AxisListType.X,
            op=mybir.AluOpType.max)
        mxb = mx_stk[:].rearrange("p (ti one) -> p ti one", one=1).to_broadcast([P, LT, E])
        ls = sb.tile([P, LE], dt.float32)
        nc.vector.tensor_tensor(
            out=ls[:].rearrange("p (ti e) -> p ti e", e=E), in0=lg3, in1=mxb,
            op=mybir.AluOpType.subtract)
        ex = sb.tile([P, LE], dt.float32)
        nc.scalar.activation(
            ex[:], ls[:], mybir.ActivationFunctionType.Exp)
        s_stk = sb.tile([P, LT], dt.float32)
        nc.vector.tensor_reduce(
            out=s_stk[:], in_=ex[:].rearrange("p (ti e) -> p ti e", e=E),
            axis=mybir.AxisListType.X, op=mybir.AluOpType.add)
        # gate into interleaved eg_stk col 2ti+1
        nc.vector.reciprocal(
            eg_stk[:].rearrange("p (ti two) -> p ti two", two=2)[:, :, 1:2],
            s_stk[:].rearrange("p (ti one) -> p ti one", one=1))
        # argmax with first-index tie-break: min over (onehot ? e : 9)
        oh = sb.tile([P, LE], dt.uint8)
        nc.vector.tensor_tensor(
            out=oh[:].rearrange("p (ti e) -> p ti e", e=E), in0=lg3, in1=mxb,
            op=mybir.AluOpType.is_equal)
        msk = sb.tile([P, LE], dt.float32)
        nc.vector.select(msk[:], oh[:], eit[:], nines[:])
        nc.vector.tensor_reduce(
            out=eg_stk[:].rearrange("p (ti two) -> p ti two", two=2)[:, :, 0:1],
            in_=msk[:].rearrange("p (ti e) -> p ti e", e=E),
            axis=mybir.AxisListType.X, op=mybir.AluOpType.min)

        if stage < 2:
            nc.compile()
            return nc

        # ---------- phase B: local positions + scatter (no collective) ----
        eidx_v = eg_stk[:].rearrange("p (ti two) -> p ti two", two=2)[:, :, 0:1]
        gate_v = eg_stk[:].rearrange("p (ti two) -> p ti two", two=2)[:, :, 1:2]
        mine_all = sb.tile([P, LE], dt.float32)      # (ti, e) columns
        nc.vector.tensor_tensor(
            out=mine_all[:].rearrange("p (ti e) -> p ti e", e=E),
            in0=eidx_v.to_broadcast([P, LT, E]),
            in1=eit[:].rearrange("p (ti e) -> p ti e", e=E),
            op=mybir.AluOpType.is_equal)

        rowf = sb.tile([P, LE], dt.float32)
        with tc.tile_pool(name="ppb", bufs=1, space="PSUM") as ppb:
            # per-(ti,e) tile sums -> [64, 1]
            pts = ppb.tile([LE, 1], dt.float32, tag="pts")
            nc.tensor.matmul(pts[:], lhsT=mine_all[:], rhs=onescol[:],
                             start=True, stop=True)
            tscol = sb.tile([LE, 1], dt.float32)
            nc.vector.tensor_copy(tscol[:], pts[:])
            # per-(ti,e) exclusive tile offsets
            poffs = ppb.tile([1, LE], dt.float32, tag="poffs")
            nc.tensor.matmul(poffs[:], lhsT=tscol[:], rhs=w64t[:],
                             start=True, stop=True)
            offsb = sb.tile([1, LE], dt.float32)
            nc.vector.tensor_copy(offsb[:], poffs[:])
            # local inclusive positions + tile offsets
            pall = ppb.tile([P, LE], dt.float32, tag="pall")
            nc.tensor.matmul(pall[:], lhsT=trit[:], rhs=mine_all[:],
                             start=True, stop=False)
            nc.tensor.matmul(pall[:], lhsT=ones1[:], rhs=offsb[:],
                             start=False, stop=True)
            # row = e*TSH + localpos - 1  (ecbt holds e*TSH - 1)
            nc.vector.tensor_tensor(
                out=rowf[:], in0=pall[:], in1=ecbt[:],
                op=mybir.AluOpType.add)
        mu8 = sb.tile([P, LE], dt.uint8)
        nc.vector.tensor_scalar(
            out=mu8[:], in0=mine_all[:], scalar1=0.5, scalar2=None,
            op0=mybir.AluOpType.is_gt)
        s1 = sb.tile([P, LE], dt.float32)
        nc.vector.select(s1[:], mu8[:], rowf[:], huget[:])
        rowmin = sb.tile([P, LT], dt.float32)
        nc.vector.tensor_reduce(
            out=rowmin[:].rearrange("p (ti one) -> p ti one", one=1),
            in_=s1[:].rearrange("p (ti e) -> p ti e", e=E),
            axis=mybir.AxisListType.X, op=mybir.AluOpType.min)
        sloti = sb.tile([P, LT], dt.int32)
        nc.vector.tensor_copy(sloti[:], rowmin[:])
        # (token_id+1, gate) pairs; interleaved columns
        pairs = sb.tile([P, LT * 2], dt.float32)
        nc.vector.tensor_copy(
            pairs[:].rearrange("p (t two) -> p t two", two=2)[:, :, 0:1],
            tokp1[:].rearrange("p (t one) -> p t one", one=1))
        nc.vector.tensor_copy(
            pairs[:].rearrange("p (t two) -> p t two", two=2)[:, :, 1:2],
            gate_v)
        for t in range(LT):
            nc.gpsimd.indirect_dma_start(
                out=igd_loc[:], out_offset=bass.IndirectOffsetOnAxis(
                    ap=sloti[:, t:t + 1], axis=0),
                in_=pairs[:, 2 * t:2 * t + 2], in_offset=None,
                bounds_check=T - 1, oob_is_err=False)
        # expert region e -> core e
        nc.gpsimd.collective_compute(
            "AllToAll", mybir.AluOpType.bypass,
            ins=[igd_loc[:]], outs=[igd_rcv[:]],
            replica_groups=[list(range(E))])

        # ---------- receive side: counts -> bases -> compaction ----------
        cntt = sb.tile([P, T // P * 2], dt.float32)  # [128, 128]
        nc.sync.dma_start(
            cntt[:], igd_rcv[:].rearrange(
                "(j lt p) two -> p j lt two", j=E, p=P))
        nzf = sb.tile([P, LE], dt.float32)           # (j, lt) columns
        nc.vector.tensor_scalar(
            out=nzf[:].rearrange("p (j lt one) -> p j lt one", j=E, one=1),
            in0=cntt[:].rearrange(
                "p (j lt two) -> p j lt two", j=E, two=2)[:, :, :, 0:1],
            scalar1=0.5, scalar2=None, op0=mybir.AluOpType.is_gt)
        nzr = sb.tile([P, E], dt.float32)
        nc.vector.tensor_reduce(
            out=nzr[:].rearrange("p (j one) -> p j one", one=1),
            in_=nzf[:].rearrange("p (j lt) -> p j lt", j=E),
            axis=mybir.AxisListType.X, op=mybir.AluOpType.add)
        bse80 = sb.tile([P, JS], dt.float32)
        with tc.tile_pool(name="ppc", bufs=1, space="PSUM") as ppc:
            pctj = ppc.tile([E, 1], dt.float32, tag="pctj")
            nc.tensor.matmul(pctj[:], lhsT=nzr[:], rhs=onescol[:],
                             start=True, stop=True)
            ctjcol = sb.tile([E, 1], dt.float32)
            nc.vector.tensor_copy(ctjcol[:], pctj[:])
            pb80 = ppc.tile([1, JS], dt.float32, tag="pb80")
            nc.tensor.matmul(pb80[:], lhsT=ctjcol[:], rhs=wtri8t[:],
                             start=True, stop=True)
            b80 = sb.tile([1, JS], dt.float32)
            nc.vector.tensor_copy(b80[:], pb80[:])
            pbs80 = ppc.tile([P, JS], dt.float32, tag="pbs80")
            nc.tensor.matmul(pbs80[:], lhsT=ones1[:], rhs=b80[:],
                             start=True, stop=True)
            nc.vector.tensor_copy(bse80[:], pbs80[:])
        gef = sb.tile([P, JS], dt.float32)           # (sc, j) columns
        nc.vector.tensor_tensor(
            out=gef[:], in0=sidxt[:], in1=bse80[:],
            op=mybir.AluOpType.is_gt)
        jcnt = sb.tile([P, SCN], dt.float32)
        nc.vector.tensor_reduce(
            out=jcnt[:].rearrange("p (sc one) -> p sc one", one=1),
            in_=gef[:].rearrange("p (sc j) -> p sc j", j=E),
            axis=mybir.AxisListType.X, op=mybir.AluOpType.add)
        bm = sb.tile([P, JS], dt.float32)
        nc.vector.tensor_tensor(
            out=bm[:], in0=gef[:], in1=bse80[:], op=mybir.AluOpType.mult)
        basejs = sb.tile([P, SCN], dt.float32)
        nc.vector.tensor_reduce(
            out=basejs[:].rearrange("p (sc one) -> p sc one", one=1),
            in_=bm[:].rearrange("p (sc j) -> p sc j", j=E),
            axis=mybir.AxisListType.X, op=mybir.AluOpType.max)
        srcf = sb.tile([P, SCN], dt.float32)
        nc.vector.tensor_scalar(
            out=srcf[:], in0=jcnt[:], scalar1=float(TSH), scalar2=None,
            op0=mybir.AluOpType.mult)
        nc.vector.tensor_tensor(
            out=srcf[:], in0=srcf[:], in1=slotdt[:], op=mybir.AluOpType.add)
        nc.vector.tensor_tensor(
            out=srcf[:], in0=srcf[:], in1=basejs[:],
            op=mybir.AluOpType.subtract)
        srci = sb.tile([P, SCN], dt.int32)
        nc.vector.tensor_copy(srci[:], srcf[:])

        # compaction + x row gathers, interleaved per slot chunk
        gxs = []
        for sc in range(SCN):
            lkg = sbr.tile([P, 2], dt.float32, tag=f"lkg{sc}")
            nc.vector.memset(lkg[:], 0.0)
            nc.gpsimd.indirect_dma_start(
                out=lkg[:], out_offset=None, in_=igd_rcv[:],
                in_offset=bass.IndirectOffsetOnAxis(
                    ap=srci[:, sc:sc + 1], axis=0),
                bounds_check=T - 1, oob_is_err=False)
            vu8 = sbr.tile([P, 1], dt.uint8, tag=f"vu{sc}")
            nc.vector.tensor_scalar(
                out=vu8[:], in0=lkg[:, 0:1], scalar1=0.5, scalar2=None,
                op0=mybir.AluOpType.is_gt)
            im1 = sbr.tile([P, 1], dt.float32, tag=f"im{sc}")
            nc.vector.tensor_scalar_add(im1[:], lkg[:, 0:1], -1.0)
            nc.vector.select(
                idxf[:, sc:sc + 1], vu8[:], im1[:], bigt[:, sc:sc + 1])
            nc.vector.tensor_copy(idx_t[:, sc:sc + 1], idxf[:, sc:sc + 1])
            nc.vector.tensor_copy(gate_f[:, sc:sc + 1], lkg[:, 1:2])
            gx = sb.tile([P, M], dt.bfloat16, tag=f"gx{sc}")
            nc.vector.memset(gx[:], 0.0)
            nc.gpsimd.indirect_dma_start(
                out=gx[:], out_offset=None, in_=xb[:],
                in_offset=bass.IndirectOffsetOnAxis(
                    ap=idx_t[:, sc:sc + 1], axis=0),
                bounds_check=T - 1, oob_is_err=False)
            gxs.append(gx)

        if stage < 3:
            nc.compile()
            return nc

        # ---------- phases C/D/E per half ----------
        with (
            tc.tile_pool(name="pstr", bufs=2, space="PSUM") as pstr,
            tc.tile_pool(name="ps1", bufs=2, space="PSUM") as ps1,
            tc.tile_pool(name="ps2", bufs=2, space="PSUM") as ps2,
        ):
            for h in range(2):
                dispT = sb.tile([P, MC * HALF], dt.bfloat16, tag="dispT")
                hT = sb.tile([P, DC * HALF], dt.bfloat16, tag="hT")
                # dispatch: transpose the gathered tiles
                for s5 in range(5):
                    sc = h * 5 + s5
                    gx = gxs[sc]
                    for mm in range(MC):
                        ptg = pstr.tile([P, P], dt.bfloat16, tag="ptg")
                        nc.tensor.transpose(
                            out=ptg[:], in_=gx[:, mm * P:(mm + 1) * P],
                            identity=idb[:])
                        nc.vector.tensor_copy(
                            dispT[:, mm * HALF + s5 * P:
                                  mm * HALF + (s5 + 1) * P],
                            ptg[:])
                # FFN1
                if stage >= 4:
                    for d in range(DC):
                        w1t = sbw1.tile([P, M], dt.bfloat16, tag="w1t")
                        nc.sync.dma_start(w1t[:], w1p[d])
                        pA = ps1.tile([P, 512], dt.float32, tag="pA")
                        pB = ps1.tile([P, P], dt.float32, tag="pB")
                        for mc in range(MC):
                            lhs = w1t[:, mc * P:(mc + 1) * P]
                            nc.tensor.matmul(
                                pA[:], lhsT=lhs,
                                rhs=dispT[:, mc * HALF:mc * HALF + 512],
                                start=(mc == 0), stop=(mc == MC - 1))
                            nc.tensor.matmul(
                                pB[:], lhsT=lhs,
                                rhs=dispT[:, mc * HALF + 512:(mc + 1) * HALF],
                                start=(mc == 0), stop=(mc == MC - 1))
                        nc.scalar.activation(
                            hT[:, d * HALF:d * HALF + 512], pA[:],
                            mybir.ActivationFunctionType.Relu,
                            bias=b1t[:, d:d + 1], scale=1.0)
                        nc.scalar.activation(
                            hT[:, d * HALF + 512:(d + 1) * HALF], pB[:],
                            mybir.ActivationFunctionType.Relu,
                            bias=b1t[:, d:d + 1], scale=1.0)
                # FFN2 + combine + scatter
                if stage >= 5:
                    for s5 in range(5):
                        sc = h * 5 + s5
                        st = sbst.tile([P, M], dt.float32, tag="st")
                        po0 = ps2.tile([P, 512], dt.float32, tag="po")
                        po1 = ps2.tile([P, 512], dt.float32, tag="po")
                        for d in range(DC):
                            lhs = hT[:, d * HALF + s5 * P:d * HALF + (s5 + 1) * P]
                            nc.tensor.matmul(
                                po0[:], lhsT=lhs,
                                rhs=w2t[:, d * M:d * M + 512],
                                start=(d == 0), stop=(d == DC - 1))
                            nc.tensor.matmul(
                                po1[:], lhsT=lhs,
                                rhs=w2t[:, d * M + 512:d * M + 1024],
                                start=(d == 0), stop=(d == DC - 1))
                        for mm, po in ((0, po0), (1, po1)):
                            nc.vector.tensor_tensor(
                                out=st[:, mm * 512:(mm + 1) * 512], in0=po[:],
                                in1=b2t[:, mm * 512:(mm + 1) * 512],
                                op=mybir.AluOpType.add)
                        nc.vector.tensor_scalar_mul(
                            st[:], st[:], gate_f[:, sc:sc + 1])
                        nc.gpsimd.indirect_dma_start(
                            out=outd[:], out_offset=bass.IndirectOffsetOnAxis(
                                ap=idx_t[:, sc:sc + 1], axis=0),
                            in_=st[:], in_offset=None,
                            bounds_check=T - 1, oob_is_err=False)

    nc.compile()
    return nc


def _make_w64():
    w = np.zeros((LE, LE), dtype=np.float32)
    for tip in range(LT):
        for ep in range(E):
            r = tip * E + ep
            for ti in range(LT):
                if tip < ti:
                    w[r, ti * E + ep] = 1.0
    return w


def _make_wtri8():
    w = np.zeros((E, JS), dtype=np.float32)
    for jp in range(E):
        for sc in range(SCN):
            for j in range(E):
                if jp < j:
                    w[jp, sc * E + j] = 1.0
    return w


def _prep_inputs(x, wg, w1, b1, w2, b2):
    bf16 = ml_dtypes.bfloat16
    tokens = np.ascontiguousarray(x.reshape(T, M)).astype(np.float32)
    xT = np.ascontiguousarray(tokens.T)
    xb = tokens.astype(bf16)
    wgf = np.ascontiguousarray(wg.astype(np.float32))
    eiota = np.broadcast_to(
        np.arange(E, dtype=np.float32), (P, LT, E)).copy()
    triu = np.triu(np.ones((P, P), dtype=np.float32))
    identf = np.eye(P, dtype=np.float32)
    identb = np.eye(P).astype(bf16)
    w64 = _make_w64()
    wtri8 = _make_wtri8()
    ecb = np.broadcast_to(
        np.tile(np.arange(E, dtype=np.float32) * TSH, LT) - 1.0,
        (P, LE)).copy()
    svals = (np.arange(SCN)[None, :] * P
             + np.arange(P)[:, None]).astype(np.float32)        # [p, sc]
    sidx = np.repeat(svals + 0.5, E, axis=1)                    # [p, (sc j)]
    slotd = svals - float(TSH)
    in_maps = []
    for e in range(E):
        w1e = np.ascontiguousarray(w1[e]).astype(bf16)          # [M, DFF]
        w1pk = np.ascontiguousarray(
            w1e.reshape(MC, P, DC, P).transpose(2, 1, 0, 3))    # [DC,P,MC,P]
        w2e = np.ascontiguousarray(w2[e]).astype(bf16)          # [DFF, M]
        w2pk = np.ascontiguousarray(
            w2e.reshape(DC, P, M).transpose(1, 0, 2))           # [P,DC,M]
        tokp1 = (e * TSH + np.arange(TSH, dtype=np.float32)
                 .reshape(LT, P).T + 1.0).copy()
        in_maps.append({
            "xTs": np.ascontiguousarray(xT[:, e * TSH:(e + 1) * TSH]),
            "xb": xb, "wg": wgf,
            "w1p": w1pk, "w2p": w2pk,
            "b1v": np.ascontiguousarray(b1[e]).astype(np.float32),
            "b2b": np.tile(np.asarray(b2[e], dtype=np.float32), (P, 1)),
            "eiota": eiota, "triu": triu,
            "identf": identf, "identb": identb,
            "w64d": w64, "wtri8d": wtri8, "ecbd": ecb,
            "sidxd": sidx, "slotdd": slotd,
            "tokp1d": tokp1,
        })
    return in_maps


def kernel(x, wg, w1, b1, w2, b2, _trace=False):
    if "nc" not in _CACHE:
        _CACHE["nc"] = _build_nc()
    nc = _CACHE["nc"]
    in_maps = _prep_inputs(
        np.asarray(x), np.asarray(wg), np.asarray(w1),
        np.asarray(b1), np.asarray(w2), np.asarray(b2))
    res = run_bass_kernel_spmd(nc, in_maps, list(range(E)), trace=_trace)
    _CACHE["last_results"] = res
    full = np.zeros((T, M), dtype=np.float32)
    for e in range(E):
        full += res.results[e]["out"]
    return full.reshape(B, S, M)
